# revision 9
# baseline (speedup 1.0000x reference)
"""Trainium2 Bass kernel for nn_MemoryEfficientGNN (2-layer hetero SAGE GNN).

Strategy (8 NeuronCores, SPMD single program):
  - Nodes dst-sharded: core c owns stay rows [c*12500,(c+1)*12500), diag rows
    [c*6250,(c+1)*6250). Edges sharded by destination, so segment sums are
    core-local (no all-reduce); full node tables rebuilt between layers with
    two AllGathers.
  - Gather of source features uses the GPSIMD dma_gather extended instruction
    (int16 indices -> source tables processed in 25000-row chunks).
  - Segment-sum uses dma_scatter_add. The SDMA CCE read-modify-write does NOT
    serialize duplicate destinations in flight (measured: full collision keeps
    ~2/128 contributions), so edges are split host-side into "rounds" with
    unique destinations per scatter instruction; rounds to the same buffer
    serialize via Tile's WAW tracking which makes accumulation exact.
  - Linear algebra on PE (fp32): per-128-row tile transposes + stacked-K
    matmul computing mean@Wl.T + h_dst@Wr.T in one instruction.
  - L2-norm / relu / LayerNorm on DVE+ACT, batched across whole slices.
"""

import os
import sys

import numpy as np

for _p in ("/opt/trn_rl_repo", "/root/.axon_site/_ro/trn_rl_repo"):
    if os.path.isdir(_p) and _p not in sys.path:
        sys.path.append(_p)

import concourse.bass as bass  # noqa: E402
import concourse.tile as tile  # noqa: E402
from concourse import bacc, mybir  # noqa: E402
import concourse.bass_utils as bass_utils  # noqa: E402
from concourse.masks import make_identity  # noqa: E402

F32 = mybir.dt.float32
I16 = mybir.dt.int16
AF = mybir.ActivationFunctionType
OP = mybir.AluOpType

NC = 8
N_STAY, N_DIAG, E = 100000, 50000, 1000000
H, F_STAY, F_DIAG, C = 64, 128, 64, 3
SS, DS = N_STAY // NC, N_DIAG // NC  # 12500, 6250
CH = 25000  # gather chunk rows (int16 index limit 32767)
LN_EPS = 1e-5
SEG = 128 * 64  # gather segment positions (8192 -> 2MB tile)
SCAT_MAX = int(os.environ.get("KSCAT_MAX", "1024"))  # max idxs per scatter_add

NW_S = (SS + 127) // 128  # 98 windows of 128 stay rows
NW_D = (DS + 127) // 128  # 49
SPAD_S = NW_S * 128  # 12544
SPAD_D = NW_D * 128  # 6272

# edge types: (name, src_table, n_src_chunks, dst_kind)
ETYPES = {
    "s2d": ("stay", 4, "diag"),
    "d2s": ("diag", 2, "stay"),
    "s2s": ("stay", 4, "stay"),
}


def _wrap_idx(a):
    """[L] -> [128, L/16] device layout (16-partition wrap, replicated x8)."""
    L = a.shape[0]
    assert L % 16 == 0
    return np.ascontiguousarray(np.tile(a.reshape(L // 16, 16).T, (8, 1)))


def _prep_edges(src, dst, n_chunks, slice_rows, dummy_row):
    """Shard edges by dst core; per (core, src-chunk) order edges by
    (occurrence-rank within dst, dst) so each round has unique destinations.

    Returns:
      seglay: per chunk: list of (seg_len, [(a, b) round-piece slices rel to seg])
      g_idx:  per chunk: [NC, L] int16 gather indices (chunk-local src)
      s_idx:  per chunk: [NC, L] int16 scatter indices (slice-local dst / dummy)
      counts: [NC, SPAD] int64 in-degree per local dst row
    """
    core = dst // slice_rows
    spad = ((slice_rows + 127) // 128) * 128
    counts = np.zeros((NC, spad), np.int64)
    per_core = []
    for c in range(NC):
        m = core == c
        s = src[m]
        d = dst[m] - c * slice_rows
        np.add.at(counts[c], d, 1)
        k = s // CH
        ls = s % CH
        chunks = []
        for kk in range(n_chunks):
            mk = k == kk
            dk, sk = d[mk], ls[mk]
            o = np.argsort(dk, kind="stable")
            dk, sk = dk[o], sk[o]
            if len(dk):
                _, fi, ct = np.unique(dk, return_index=True, return_counts=True)
                r = np.arange(len(dk)) - np.repeat(fi, ct)
                o2 = np.argsort(r, kind="stable")
                chunks.append((dk[o2], sk[o2], r[o2]))
            else:
                chunks.append((dk, sk, dk))
        per_core.append(chunks)

    seglay, g_idx, s_idx = [], [], []
    for kk in range(n_chunks):
        n_rounds = max(
            (int(per_core[c][kk][2].max()) + 1 if len(per_core[c][kk][2]) else 0)
            for c in range(NC)
        )
        sizes = []
        for r in range(n_rounds):
            mx = max(int((per_core[c][kk][2] == r).sum()) for c in range(NC))
            sizes.append(((mx + 127) // 128) * 128)
        L = sum(sizes)
        gi = np.zeros((NC, L), np.int16)
        si = np.full((NC, L), dummy_row, np.int16)
        offs = np.cumsum([0] + sizes)
        for c in range(NC):
            dk, sk, r = per_core[c][kk]
            for rr in range(n_rounds):
                mrr = r == rr
                n = int(mrr.sum())
                gi[c, offs[rr] : offs[rr] + n] = sk[mrr]
                si[c, offs[rr] : offs[rr] + n] = dk[mrr]
        # segment layout: greedy pack positions into <= SEG, round pieces split
        segs = []
        pos = 0
        while pos < L:
            seg_len = min(SEG, L - pos)
            pieces = []
            for rr in range(n_rounds):
                a = max(offs[rr], pos)
                b = min(offs[rr + 1], pos + seg_len)
                while b > a:
                    e = min(a + SCAT_MAX, b)
                    pieces.append((a - pos, e - pos))
                    a = e
            segs.append((seg_len, pieces))
            pos += seg_len
        seglay.append(segs)
        g_idx.append(gi)
        s_idx.append(si)
    return seglay, g_idx, s_idx, counts


def _build(nc_obj, meta):
    """Emit the Tile program. meta: dict with seglay per etype + flags."""
    nc = nc_obj
    t_in = {}

    def inp(name, shape, dt=F32):
        t_in[name] = nc.dram_tensor(name, list(shape), dt, kind="ExternalInput")
        return t_in[name]

    xs = inp("xs", [SPAD_S, F_STAY])
    xd = inp("xd", [SPAD_D, F_DIAG])
    wps = inp("wps", [F_STAY, H])
    wpd = inp("wpd", [F_DIAG, H])
    wc = inp("wc", [H, C])
    lng = {}
    for l in range(2):
        lng[l] = (inp(f"g{l}", [128, H]), inp(f"b{l}", [128, H]))
    rhs_t = {}
    for l, t in meta["convs_all"]:
        rhs_t[(l, t)] = inp(f"rhs{l}{t}", [128, H])
    rc_t = {
        "s2d": inp("rc_s2d", [128, NW_D]),
        "d2s": inp("rc_d2s", [128, NW_S]),
        "s2s": inp("rc_s2s", [128, NW_S]),
    }
    gi_t, si_t = {}, {}
    for t, (srcn, nch, dstk) in ETYPES.items():
        for k in range(nch):
            L = meta["L"][t][k]
            gi_t[(t, k)] = inp(f"gi_{t}{k}", [128, L // 16], I16)
            si_t[(t, k)] = inp(f"si_{t}{k}", [128, L // 16], I16)

    out_t = nc.dram_tensor("out", [SS, C], F32, kind="ExternalOutput")

    # internal DRAM
    h_stay = nc.dram_tensor("h_stay", [N_STAY, H], F32, addr_space="Shared")
    h_diag = nc.dram_tensor("h_diag", [N_DIAG, H], F32, addr_space="Shared")
    hs_loc = nc.dram_tensor("hs_loc", [SPAD_S, H], F32)
    hd_loc = nc.dram_tensor("hd_loc", [SPAD_D, H], F32)
    S_buf = {
        "s2d": nc.dram_tensor("S_s2d", [SPAD_D + 128, H], F32),
        "d2s": nc.dram_tensor("S_d2s", [SPAD_S + 128, H], F32),
        "s2s": nc.dram_tensor("S_s2s", [SPAD_S + 128, H], F32),
    }
    tables = {"stay": h_stay, "diag": h_diag}

    rg = [list(range(NC))]

    with tile.TileContext(nc) as tc:
        with (
            tc.tile_pool(name="const", bufs=1) as cpool,
            tc.tile_pool(name="seg", bufs=2) as segpool,
            tc.tile_pool(name="idx", bufs=3) as ipool,
            tc.tile_pool(name="slab", bufs=1) as slabpool,
            tc.tile_pool(name="work", bufs=3) as wpool,
            tc.tile_pool(name="small", bufs=4) as spool,
            tc.tile_pool(name="mm", bufs=2, space="PSUM") as ppool,
            tc.tile_pool(name="tr", bufs=2, space="PSUM") as tpool,
        ):
            ident = cpool.tile([128, 128], F32, tag="ident")
            make_identity(nc, ident[:])
            ztile = cpool.tile([128, 2048], F32, tag="ztile")
            nc.vector.memset(ztile[:], 0.0)

            def load_const(t, shape, tag):
                tl = cpool.tile(list(shape), F32, tag=tag)
                nc.sync.dma_start(out=tl[:], in_=t[:])
                return tl

            wps_s = load_const(wps, [F_STAY, H], "wps")
            wpd_s = load_const(wpd, [F_DIAG, H], "wpd")
            wc_s = load_const(wc, [H, C], "wc")
            lng_s = {
                l: (
                    load_const(lng[l][0], [128, H], f"g{l}"),
                    load_const(lng[l][1], [128, H], f"b{l}"),
                )
                for l in range(2)
            }
            rhs_s = {
                key: load_const(rhs_t[key], [128, H], f"rhs{key[0]}{key[1]}")
                for key in rhs_t
            }
            rc_s = {
                t: load_const(rc_t[t], [128, NW_D if t == "s2d" else NW_S], f"rc{t}")
                for t in rc_t
            }

            # ---------- helpers ----------
            def zero_dram(dram, rows):
                nwnd = rows // 128
                done = 0
                while done < nwnd:
                    n = min(nwnd - done, 32)
                    nc.sync.dma_start(
                        out=dram[done * 128 : (done + n) * 128, :].rearrange(
                            "(w p) h -> p w h", p=128
                        ),
                        in_=ztile[:, : n * H].rearrange("p (w h) -> p w h", h=H),
                    )
                    done += n

            def proj(x_dram, w_sb, nw, f_in, stage):
                """h = relu(x @ W.T) tile-by-tile into staging slab."""
                for w in range(nw):
                    xt = wpool.tile([128, f_in], F32, tag="projx")
                    nc.sync.dma_start(
                        out=xt[:], in_=x_dram[w * 128 : (w + 1) * 128, :]
                    )
                    tp = tpool.tile([f_in, 128], F32, tag="tr")
                    nc.tensor.transpose(tp[:], xt[:], ident[:])
                    xtt = wpool.tile([f_in, 128], F32, tag="projxt")
                    nc.scalar.copy(xtt[:], tp[:])
                    ps = ppool.tile([128, H], F32, tag="mmout")
                    nc.tensor.matmul(ps[:], lhsT=xtt[:], rhs=w_sb[:], start=True, stop=True)
                    nc.scalar.activation(stage[:, w, :], ps[:], AF.Relu)

            def conv_scatter(l, t):
                srcn, nch, dstk = ETYPES[t]
                table = tables[srcn]
                for k in range(nch):
                    base = k * CH
                    pos = 0
                    for (seg_len, pieces) in meta["seglay"][t][k]:
                        st = segpool.tile([128, SEG // 128, H], F32, tag="seg")
                        git = ipool.tile([128, SEG // 16], I16, tag="gidx")
                        sit = ipool.tile([128, SEG // 16], I16, tag="sidx")
                        c0, c1 = pos // 16, (pos + seg_len) // 16
                        nc.sync.dma_start(out=git[:, : seg_len // 16], in_=gi_t[(t, k)][:, c0:c1])
                        nc.sync.dma_start(out=sit[:, : seg_len // 16], in_=si_t[(t, k)][:, c0:c1])
                        nc.gpsimd.dma_gather(
                            st[:, : seg_len // 128, :],
                            table[base : base + CH, :],
                            git[:, : seg_len // 16],
                            num_idxs=seg_len,
                            num_idxs_reg=seg_len,
                            elem_size=H,
                            queue_num=0,
                            single_packet=False,
                        )
                        for (a, b) in pieces:
                            nc.gpsimd.dma_scatter_add(
                                S_buf[t][:, :],
                                st[:, a // 128 : b // 128, :],
                                sit[:, a // 16 : b // 16],
                                num_idxs=b - a,
                                num_idxs_reg=b - a,
                                elem_size=H,
                                queue_num=0,
                                single_packet=False,
                            )
                        pos += seg_len

            def load_slab(dram, nw, tag):
                sl = slabpool.tile([128, nw, H], F32, tag=tag)
                nc.gpsimd.dma_start(
                    out=sl[:],
                    in_=dram[: nw * 128, :].rearrange("(w p) h -> p w h", p=128),
                )
                return sl

            def l2norm(o_ps, dst_ap, w):
                """dst = o / max(||o||, eps), o in PSUM [128, H]."""
                sq = wpool.tile([128, H], F32, tag="sq")
                n2 = spool.tile([128, 1], F32, tag="n2")
                nc.scalar.activation(sq[:], o_ps[:], AF.Square, accum_out=n2[:])
                n2e = spool.tile([128, 1], F32, tag="n2e")
                nc.vector.tensor_scalar(n2e[:], n2[:], 1e-24, None, OP.add)
                rn = spool.tile([128, 1], F32, tag="rn")
                nc.vector.reciprocal(rn[:], n2e[:])
                rno = spool.tile([128, 1], F32, tag="rno")
                nc.scalar.activation(rno[:], rn[:], AF.Sqrt)
                nc.vector.tensor_scalar(dst_ap, o_ps[:], rno[:, 0:1], None, OP.mult)

            def batched_ln(stage, scratch, nw, l, out_dram):
                """LayerNorm over H for stage [128, nw, H]; writes out_dram."""
                mu = spool.tile([128, nw], F32, tag="mu")
                nc.vector.tensor_reduce(mu[:], stage[:], mybir.AxisListType.X, OP.add)
                nc.scalar.activation(mu[:], mu[:], AF.Copy, scale=1.0 / H)
                nc.vector.tensor_tensor(
                    stage[:], stage[:], mu[:, :, None].to_broadcast([128, nw, H]), OP.subtract
                )
                nc.scalar.activation(scratch[:, :nw, :], stage[:], AF.Square)
                var = spool.tile([128, nw], F32, tag="var")
                nc.vector.tensor_reduce(var[:], scratch[:, :nw, :], mybir.AxisListType.X, OP.add)
                nc.vector.tensor_scalar(var[:], var[:], 1.0 / H, LN_EPS, OP.mult, OP.add)
                nc.vector.reciprocal(var[:], var[:])
                nc.scalar.activation(var[:], var[:], AF.Sqrt)
                nc.vector.tensor_tensor(
                    stage[:], stage[:], var[:, :, None].to_broadcast([128, nw, H]), OP.mult
                )
                if meta["apply_ln_affine"]:
                    g_sb, b_sb = lng_s[l]
                    nc.vector.tensor_tensor(
                        stage[:], stage[:], g_sb[:, None, :].to_broadcast([128, nw, H]), OP.mult
                    )
                    nc.vector.tensor_tensor(
                        stage[:], stage[:], b_sb[:, None, :].to_broadcast([128, nw, H]), OP.add
                    )
                nc.sync.dma_start(
                    out=out_dram[: nw * 128, :].rearrange("(w p) h -> p w h", p=128),
                    in_=stage[:],
                )

            def transpose_to(src_ap, dst_sb_ap, tag):
                tp = tpool.tile([H, 128], F32, tag=tag)
                nc.tensor.transpose(tp[:], src_ap, ident[:])
                nc.scalar.copy(dst_sb_ap, tp[:])

            # ---------- phase 0: projections ----------
            hs_stage = slabpool.tile([128, NW_S, H], F32, tag="hs_stage")
            hd_stage = slabpool.tile([128, NW_D, H], F32, tag="hd_stage")
            proj(xs, wps_s, NW_S, F_STAY, hs_stage)
            proj(xd, wpd_s, NW_D, F_DIAG, hd_stage)
            nc.sync.dma_start(
                out=hs_loc[:, :].rearrange("(w p) h -> p w h", p=128), in_=hs_stage[:]
            )
            nc.sync.dma_start(
                out=hd_loc[:, :].rearrange("(w p) h -> p w h", p=128), in_=hd_stage[:]
            )
            nc.gpsimd.collective_compute(
                "AllGather", OP.bypass, replica_groups=rg,
                ins=[hs_loc[0:SS, :]], outs=[h_stay[:, :]],
            )
            nc.gpsimd.collective_compute(
                "AllGather", OP.bypass, replica_groups=rg,
                ins=[hd_loc[0:DS, :]], outs=[h_diag[:, :]],
            )

            # ---------- layers ----------
            for l in range(2):
                convs = ["s2d", "d2s", "s2s"] if l == 0 else ["d2s", "s2s"]
                for t in convs:
                    rows = SPAD_D + 128 if t == "s2d" else SPAD_S + 128
                    zero_dram(S_buf[t], rows)
                if not meta.get("skip_scatter"):
                    for t in convs:
                        conv_scatter(l, t)

                # ----- stay post: o_s1 (d2s), o_s2 (s2s) -----
                S1 = load_slab(S_buf["d2s"], NW_S, "S1")
                S2 = load_slab(S_buf["s2s"], NW_S, "S2")
                hd_sl = load_slab(hs_loc, NW_S, "hdst_stay")
                nc.vector.tensor_tensor(
                    S1[:], S1[:], rc_s["d2s"][:, :, None].to_broadcast([128, NW_S, H]), OP.mult
                )
                nc.vector.tensor_tensor(
                    S2[:], S2[:], rc_s["s2s"][:, :, None].to_broadcast([128, NW_S, H]), OP.mult
                )
                if l == 0:
                    S3 = load_slab(S_buf["s2d"], NW_D, "S3")
                    hdd_sl = load_slab(hd_loc, NW_D, "hdst_diag")
                    nc.vector.tensor_tensor(
                        S3[:], S3[:], rc_s["s2d"][:, :, None].to_broadcast([128, NW_D, H]), OP.mult
                    )

                for w in range(0 if meta.get("skip_post") else NW_S):
                    stackA = wpool.tile([128, 128], F32, tag="stackA")
                    stackB = wpool.tile([128, 128], F32, tag="stackB")
                    transpose_to(S1[:, w, :], stackA[0:H, :], "tr")
                    transpose_to(S2[:, w, :], stackB[0:H, :], "tr")
                    tp3 = tpool.tile([H, 128], F32, tag="tr")
                    nc.tensor.transpose(tp3[:], hd_sl[:, w, :], ident[:])
                    nc.scalar.copy(stackA[H:128, :], tp3[:])
                    nc.scalar.copy(stackB[H:128, :], tp3[:])
                    o1 = ppool.tile([128, H], F32, tag="mmout")
                    o2 = ppool.tile([128, H], F32, tag="mmout")
                    nc.tensor.matmul(o1[:], lhsT=stackA[:], rhs=rhs_s[(l, "d2s")][:], start=True, stop=True)
                    nc.tensor.matmul(o2[:], lhsT=stackB[:], rhs=rhs_s[(l, "s2s")][:], start=True, stop=True)
                    o1n = wpool.tile([128, H], F32, tag="o1n")
                    l2norm(o1, o1n[:], w)
                    l2norm(o2, S2[:, w, :], w)
                    nc.vector.tensor_tensor(o1n[:], o1n[:], S2[:, w, :], OP.add)
                    # relu((o1+o2)/2) -> write into S1 slab (stay stage)
                    nc.scalar.activation(S1[:, w, :], o1n[:], AF.Relu, scale=0.5)
                batched_ln(S1, S2, NW_S, l, hs_loc)

                if l == 0:
                    for w in range(0 if meta.get("skip_post") else NW_D):
                        stackC = wpool.tile([128, 128], F32, tag="stackC")
                        transpose_to(S3[:, w, :], stackC[0:H, :], "tr")
                        transpose_to(hdd_sl[:, w, :], stackC[H:128, :], "tr")
                        o3 = ppool.tile([128, H], F32, tag="mmout")
                        nc.tensor.matmul(o3[:], lhsT=stackC[:], rhs=rhs_s[(0, "s2d")][:], start=True, stop=True)
                        o3n = wpool.tile([128, H], F32, tag="o1n")
                        l2norm(o3, o3n[:], w)
                        nc.scalar.activation(S3[:, w, :], o3n[:], AF.Relu)
                    batched_ln(S3, hdd_sl, NW_D, 0, hd_loc)
                    nc.gpsimd.collective_compute(
                        "AllGather", OP.bypass, replica_groups=rg,
                        ins=[hs_loc[0:SS, :]], outs=[h_stay[:, :]],
                    )
                    nc.gpsimd.collective_compute(
                        "AllGather", OP.bypass, replica_groups=rg,
                        ins=[hd_loc[0:DS, :]], outs=[h_diag[:, :]],
                    )

            # ---------- classifier: out = h_stay2 @ Wc.T ----------
            ostage = slabpool.tile([128, NW_S, C], F32, tag="ostage")
            # final stay activations are in hs_loc (written by last batched_ln)
            h2 = load_slab(hs_loc, NW_S, "S2")
            for w in range(NW_S):
                h2t = wpool.tile([H, 128], F32, tag="h2t")
                transpose_to(h2[:, w, :], h2t[:], "tr")
                oc = ppool.tile([128, C], F32, tag="mmout")
                nc.tensor.matmul(oc[:], lhsT=h2t[:], rhs=wc_s[:], start=True, stop=True)
                nc.scalar.copy(ostage[:, w, :], oc[:])
            nfull = SS // 128  # 97 full windows
            nc.sync.dma_start(
                out=out_t[: nfull * 128, :].rearrange("(w p) c -> p w c", p=128),
                in_=ostage[:, :nfull, :],
            )
            tail = SS - nfull * 128  # 84
            nc.sync.dma_start(
                out=out_t[nfull * 128 :, :], in_=ostage[:tail, nfull, :]
            )

    nc.compile()
    return nc


_CACHE = {}


def kernel(**inputs):
    x_stay = np.asarray(inputs["x_stay"], np.float32)
    x_diag = np.asarray(inputs["x_diag"], np.float32)
    Wl = np.asarray(inputs["Wl"], np.float32)
    bl = np.asarray(inputs["bl"], np.float32)
    Wr = np.asarray(inputs["Wr"], np.float32)
    ln_g = np.asarray(inputs["ln_g"], np.float32)
    ln_b = np.asarray(inputs["ln_b"], np.float32)
    Wc = np.asarray(inputs["Wc"], np.float32)
    bc = np.asarray(inputs["bc"], np.float32)
    bp_stay = np.asarray(inputs["bp_stay"], np.float32)
    bp_diag = np.asarray(inputs["bp_diag"], np.float32)

    assert np.all(bl == 0) and np.all(bc == 0), "nonzero conv/cls bias unsupported"
    assert np.all(bp_stay == 0) and np.all(bp_diag == 0), "nonzero proj bias unsupported"

    # ---- host preprocessing: edge sharding ----
    ed = {}
    counts = {}
    for t, key in [("s2d", ("s2d_src", "s2d_dst")), ("d2s", ("d2s_src", "d2s_dst")),
                   ("s2s", ("s2s_src", "s2s_dst"))]:
        srcn, nch, dstk = ETYPES[t]
        slice_rows = DS if dstk == "diag" else SS
        dummy = SPAD_D if dstk == "diag" else SPAD_S
        src = np.asarray(inputs[key[0]]).astype(np.int64)
        dst = np.asarray(inputs[key[1]]).astype(np.int64)
        seglay, g_idx, s_idx, cnt = _prep_edges(src, dst, nch, slice_rows, dummy)
        ed[t] = (seglay, g_idx, s_idx)
        counts[t] = cnt

    meta = {
        "skip_scatter": bool(int(os.environ.get("KSKIP_SCATTER", "0"))),
        "skip_post": bool(int(os.environ.get("KSKIP_POST", "0"))),
        "seglay": {t: ed[t][0] for t in ed},
        "L": {t: [ed[t][1][k].shape[1] for k in range(ETYPES[t][1])] for t in ed},
        "convs_all": [(l, t) for l in range(2) for t in (["s2d", "d2s", "s2s"] if l == 0 else ["d2s", "s2s"])],
        "apply_ln_affine": not (np.all(ln_g == 1) and np.all(ln_b == 0)),
    }

    cache_key = repr((meta["skip_scatter"], meta["skip_post"])) + repr(meta["L"]) + repr([[s[0] for s in meta["seglay"][t][k]] for t in meta["seglay"] for k in range(len(meta["seglay"][t]))])
    if cache_key not in _CACHE:
        nc = bacc.Bacc("TRN2", target_bir_lowering=False, debug=False, num_devices=NC)
        _CACHE[cache_key] = _build(nc, meta)
    nc = _CACHE[cache_key]

    # ---- per-core input maps ----
    in_maps = []
    for c in range(NC):
        m = {}
        xs = np.zeros((SPAD_S, F_STAY), np.float32)
        xs[:SS] = x_stay[c * SS : (c + 1) * SS]
        xd = np.zeros((SPAD_D, F_DIAG), np.float32)
        xd[:DS] = x_diag[c * DS : (c + 1) * DS]
        m["xs"], m["xd"] = xs, xd
        m["wps"] = np.ascontiguousarray(np.asarray(inputs["Wp_stay"], np.float32).T)
        m["wpd"] = np.ascontiguousarray(np.asarray(inputs["Wp_diag"], np.float32).T)
        m["wc"] = np.ascontiguousarray(Wc.T)
        for l in range(2):
            m[f"g{l}"] = np.ascontiguousarray(np.tile(ln_g[l][None, :], (128, 1)))
            m[f"b{l}"] = np.ascontiguousarray(np.tile(ln_b[l][None, :], (128, 1)))
        for l, t in meta["convs_all"]:
            ti = {"s2d": 0, "d2s": 1, "s2s": 2}[t]
            m[f"rhs{l}{t}"] = np.ascontiguousarray(
                np.concatenate([Wl[l, ti].T, Wr[l, ti].T], axis=0)
            )
        for t in ETYPES:
            nw = NW_D if ETYPES[t][2] == "diag" else NW_S
            spad = nw * 128
            rc = 1.0 / np.maximum(counts[t][c][:spad], 1).astype(np.float32)
            m[f"rc_{t}"] = np.ascontiguousarray(rc.reshape(nw, 128).T)
        for t in ETYPES:
            for k in range(ETYPES[t][1]):
                m[f"gi_{t}{k}"] = _wrap_idx(ed[t][1][k][c])
                m[f"si_{t}{k}"] = _wrap_idx(ed[t][2][k][c])
        in_maps.append(m)

    import time as _time
    _t0 = _time.time()
    try:
        res = bass_utils.run_bass_kernel_spmd(
            nc, in_maps, core_ids=list(range(NC)),
            trace=bool(int(os.environ.get("KTRACE", "0"))),
        )
    except ModuleNotFoundError:
        res = bass_utils.run_bass_kernel_spmd(nc, in_maps, core_ids=list(range(NC)))
    kernel.last_exec_wall_s = _time.time() - _t0
    if res.exec_time_ns is not None:
        print(f"HW exec time: {res.exec_time_ns} ns")
    out = np.concatenate([res.results[c]["out"] for c in range(NC)], axis=0)
    return out.astype(np.float32)
